# revision 26
# baseline (speedup 1.0000x reference)
"""Causal multi-head attention (B=4, T=2048, C=384, 6 heads of 64) on 8 trn2 cores.

Sharding: 24 (batch, head) pairs -> 8 cores; core c handles batch c//2 and
heads [3*(c%2), 3*(c%2)+3). Each core computes q/k/v projections for its 3
heads, causal softmax(q k^T / 8) v, and a PARTIAL output projection
ctx_heads @ Wo_heads. Host sums the two partials per batch and adds the
exactly-folded bias terms (bv @ Wo + bo; softmax weights sum to 1 so a v-bias
contributes bv @ Wo to every row).

Requires bq == bk == 0 (true for this problem: spec fill=zeros).
"""

import math
import os
from contextlib import ExitStack

import numpy as np

B, T, C = 4, 2048, 384
NH, D = 6, 64          # total heads, head dim
HPC = 3                # heads per core
NCORES = 8
NKC = C // 128         # 3 contraction chunks for the projections
NTB = T // 128         # 16 row blocks
TCW = 512              # t-chunk width for the attention loop
NTC = T // TCW         # 4 t-chunks

_CACHED_NC = None


def build_nc():
    import concourse.bass as bass
    import concourse.mybir as mybir
    import concourse.tile as tile
    from concourse import bacc

    F32 = mybir.dt.float32
    F32R = mybir.dt.float32r
    EXPF = mybir.ActivationFunctionType.Exp

    nc = bacc.Bacc("TRN2", target_bir_lowering=False, debug=False)

    xt = nc.dram_tensor("xt", [C, T], F32R, kind="ExternalInput")
    wqk = nc.dram_tensor("wqk", [C, 512], F32R, kind="ExternalInput")
    wv = nc.dram_tensor("wv", [C, 256], F32R, kind="ExternalInput")
    wo = nc.dram_tensor("wo", [HPC * D, 384], F32R, kind="ExternalInput")
    zt = nc.dram_tensor("zt", [128, 512], F32R, kind="ExternalInput")
    vones = nc.dram_tensor("vones", [128, NTB * HPC * 2], F32R, kind="ExternalInput")
    out = nc.dram_tensor("out", [T, C], F32, kind="ExternalOutput")
    srow = nc.dram_tensor("srow", [NTC * HPC, TCW], F32)  # sums rows scratch
    rrow = nc.dram_tensor("rrow", [NTC * HPC, TCW], F32)  # recip rows scratch
    debug = bool(os.environ.get("KBG_DEBUG"))
    if debug:
        d_qt = nc.dram_tensor("d_qt", [4, 128, T], F32R, kind="ExternalOutput")
        d_v = nc.dram_tensor("d_v", [128, NTB, 3 * 66], F32R, kind="ExternalOutput")
        d_e = nc.dram_tensor("d_e", [128, 1024], F32R, kind="ExternalOutput")
        d_eraw = nc.dram_tensor("d_eraw", [128, 1024], F32R, kind="ExternalOutput")
        d_ctxT = nc.dram_tensor("d_ctxT", [HPC, 64, T], F32R, kind="ExternalOutput")
        d_rec = nc.dram_tensor("d_rec", [64, TCW], F32, kind="ExternalOutput")
        d_cps = nc.dram_tensor("d_cps", [66, TCW], F32, kind="ExternalOutput")
        d_scr = nc.dram_tensor("d_scr", [1, TCW], F32, kind="ExternalOutput")

    with ExitStack() as ctx:
        tc = ctx.enter_context(tile.TileContext(nc))
        const = ctx.enter_context(tc.tile_pool(name="const", bufs=1))
        xpool = ctx.enter_context(tc.tile_pool(name="xp", bufs=1))
        qkpool = ctx.enter_context(tc.tile_pool(name="qkp", bufs=1))
        vpool = ctx.enter_context(tc.tile_pool(name="vp", bufs=1))
        expp = ctx.enter_context(tc.tile_pool(name="expp", bufs=4))
        cxp = ctx.enter_context(tc.tile_pool(name="cxp", bufs=1))
        rpool = ctx.enter_context(tc.tile_pool(name="rp", bufs=4))
        ps_s = ctx.enter_context(tc.tile_pool(name="ps_s", bufs=2, space="PSUM"))
        ps_c = ctx.enter_context(tc.tile_pool(name="ps_c", bufs=3, space="PSUM"))
        ps_o = ctx.enter_context(tc.tile_pool(name="ps_o", bufs=1, space="PSUM"))

        # ---- constants ----
        wqk_sb = []
        wv_sb = []
        for kc in range(NKC):
            w1 = const.tile([128, 512], F32R, tag=f"wqk{kc}")
            nc.sync.dma_start(out=w1, in_=wqk[kc * 128:(kc + 1) * 128, :])
            wqk_sb.append(w1)
            w2 = const.tile([128, 256], F32R, tag=f"wv{kc}")
            nc.sync.dma_start(out=w2, in_=wv[kc * 128:(kc + 1) * 128, :])
            wv_sb.append(w2)
        wo_sb = []
        for h in range(HPC):
            w3 = const.tile([64, 384], F32R, tag=f"wo{h}")
            nc.sync.dma_start(out=w3, in_=wo[h * 64:(h + 1) * 64, :])
            wo_sb.append(w3)
        zt_sb = const.tile([128, 512], F32R, tag="zt")
        nc.sync.dma_start(out=zt_sb, in_=zt[:, :])

        # ---- x^T in SBUF, as 12 [128, 512] chunks for fine-grained deps ----
        xts = [[None] * 4 for _ in range(NKC)]
        for kc in range(NKC):
            for nch in range(4):
                t_ = xpool.tile([128, 512], F32R, tag=f"xt{kc}_{nch}")
                nc.sync.dma_start(
                    out=t_,
                    in_=xt[kc * 128:(kc + 1) * 128, nch * 512:(nch + 1) * 512],
                )
                xts[kc][nch] = t_

        # ---- projections: qT/kT packed [d(2 heads), T] ----
        # wqk columns: mt0=[q0|q1] mt1=[k0|k1] mt2=[q2|q2] mt3=[k2|k2]
        # (q columns pre-scaled by 1/sqrt(D) on host)
        qk_names = ["qT01", "kT01", "qT22", "kT22"]
        qkT = {}
        for mt, name in enumerate(qk_names):
            qkT[name] = qkpool.tile([128, T], F32R, tag=name, name=name)
        for mt, name in enumerate(qk_names):
            for nch in range(4):
                ps = ps_s.tile([128, 1024], F32, tag="S")
                for kc in range(NKC):
                    nc.tensor.matmul(
                        ps[:, 0:512],
                        lhsT=wqk_sb[kc][:, mt * 128:(mt + 1) * 128],
                        rhs=xts[kc][nch][:, :],
                        start=(kc == 0),
                        stop=(kc == NKC - 1),
                    )
                nc.any.tensor_copy(
                    out=qkT[name][:, nch * 512:(nch + 1) * 512], in_=ps[:, 0:512]
                )

        # ---- v in natural [s, d'] layout + ones column per head (65 wide) ----
        v_sb = vpool.tile([128, NTB, 3 * 66], F32R, tag="vsb")
        ones_view = v_sb.rearrange("p s (h e) -> p s h e", e=66)[:, :, :, 64:66]
        nc.sync.dma_start(
            out=ones_view, in_=vones.rearrange("p (s h o) -> p s h o", h=HPC, o=2)
        )
        for tb in range(NTB):
            ps = ps_s.tile([128, 1024], F32, tag="S")
            for kc in range(NKC):
                nc.tensor.matmul(
                    ps[:, 0:256],
                    lhsT=xts[kc][tb // 4][:, (tb % 4) * 128:(tb % 4 + 1) * 128],
                    rhs=wv_sb[kc][:, :],
                    start=(kc == 0),
                    stop=(kc == NKC - 1),
                )
            dst = v_sb[:, tb, :].rearrange("p (h e) -> p h e", e=66)[:, :, 0:64]
            src = ps[:, 0:192].rearrange("p (h e) -> p h e", e=64)
            nc.any.tensor_copy(out=dst, in_=src)

        # ---- per-head normalized ctx^T [64, T] ----
        ctxT = [
            cxp.tile([64, T], F32R, tag=f"ctxT{h}", name=f"ctxT{h}")
            for h in range(HPC)
        ]

        def mask_exp_block(e_tile, col0, sbm):
            """Causal-mask the exp'd S^T block at e_tile[:, col0:col0+512].

            sbm = s_block_start - t_chunk_start (>= 0 on diagonal blocks).
            Cols [col0, col0+sbm) are fully above the diagonal (-> *0) and
            cols [col0+sbm, col0+sbm+128) are triangular; zt_sb is laid out
            as [384 zeros | 128-wide triangle] so one suffix slice covers
            both regions in a single multiply."""
            w = sbm + 128
            nc.vector.tensor_mul(
                e_tile[:, col0:col0 + w],
                e_tile[:, col0:col0 + w],
                zt_sb[:, 512 - w:512],
            )

        for tci in range(NTC):
            tsl = slice(tci * TCW, (tci + 1) * TCW)
            nsb = 4 * tci + 4
            cps = [
                ps_c.tile([128, TCW], F32, tag="ctx", name=f"cps{tci}_{h}")
                for h in range(HPC)
            ]
            for sb in range(nsb):
                sbm = sb * 128 - tci * TCW  # >= 0 on diagonal blocks
                # --- heads 0,1: row-tiled pair (K=64 each) ---
                s01 = ps_s.tile([128, 1024], F32, tag="S")
                e01 = expp.tile([128, 1024], F32R, tag="E")
                for hh in range(2):
                    psl = slice(hh * 64, (hh + 1) * 64)
                    nc.tensor.matmul(
                        s01[:, hh * 512:(hh + 1) * 512],
                        lhsT=qkT["kT01"][psl, sb * 128:(sb + 1) * 128],
                        rhs=qkT["qT01"][psl, tsl],
                        start=True,
                        stop=True,
                    )
                nc.scalar.activation(e01[:, :], s01[:, :], EXPF)
                if debug and tci == 0 and sb == 0:
                    nc.sync.dma_start(out=d_eraw[:, :], in_=e01[:, :])
                if sbm >= 0:
                    mask_exp_block(e01, 0, sbm)
                    mask_exp_block(e01, 512, sbm)
                if debug and tci == 0 and sb == 0:
                    nc.sync.dma_start(out=d_e[:, :], in_=e01[:, :])
                for hh in range(2):
                    nc.tensor.matmul(
                        cps[hh][0:66, :],
                        lhsT=v_sb[:, sb, hh * 66:(hh + 1) * 66],
                        rhs=e01[:, hh * 512:(hh + 1) * 512],
                        start=(sb == 0),
                        stop=(sb == nsb - 1),
                    )
                # --- head 2: packed across (sb, sb+1) pairs ---
                if sb % 2 == 0:
                    s2 = ps_s.tile([128, 1024], F32, tag="S")
                    e2 = expp.tile([128, 1024], F32R, tag="E")
                    for j in range(2):
                        sbj = sb + j
                        psl = slice(j * 64, (j + 1) * 64)
                        nc.tensor.matmul(
                            s2[:, j * 512:(j + 1) * 512],
                            lhsT=qkT["kT22"][psl, sbj * 128:(sbj + 1) * 128],
                            rhs=qkT["qT22"][psl, tsl],
                            start=True,
                            stop=True,
                        )
                    nc.scalar.activation(e2[:, :], s2[:, :], EXPF)
                    for j in range(2):
                        sbmj = (sb + j) * 128 - tci * TCW
                        if sbmj >= 0:
                            mask_exp_block(e2, j * 512, sbmj)
                    for j in range(2):
                        nc.tensor.matmul(
                            cps[2][0:66, :],
                            lhsT=v_sb[:, sb + j, 132:198],
                            rhs=e2[:, j * 512:(j + 1) * 512],
                            start=(sb + j == 0),
                            stop=(sb + j == nsb - 1),
                        )
            # --- normalize: ctxT_n = ctx / sums (sums = row 64 of cps) ---
            if debug and tci == 0:
                cpy = rpool.tile([66, TCW], F32, tag="dcpy", name=f"dcpy{tci}")
                nc.vector.tensor_copy(out=cpy[:, :], in_=cps[0][0:66, :])
                nc.sync.dma_start(out=d_cps[:, :], in_=cpy[:, :])
            # --- normalize: evac unnormalized ctx+sums, batched reciprocal ---
            cues = []
            for h in range(HPC):
                cue = rpool.tile([66, TCW], F32, tag=f"cue{h}", name=f"cue{tci}_{h}")
                nc.vector.tensor_copy(out=cue[:, :], in_=cps[h][0:66, :])
                nc.sync.dma_start(
                    out=srow[tci * HPC + h:tci * HPC + h + 1, :], in_=cue[64:65, :]
                )
                cues.append(cue)
            sall = rpool.tile([HPC, TCW], F32, tag="sall", name=f"sall{tci}")
            nc.sync.dma_start(
                out=sall[:, :], in_=srow[tci * HPC:(tci + 1) * HPC, :]
            )
            rall = rpool.tile([HPC, TCW], F32, tag="rall", name=f"rall{tci}")
            nc.vector.reciprocal(out=rall[:, :], in_=sall[:, :])
            nc.sync.dma_start(
                out=rrow[tci * HPC:(tci + 1) * HPC, :], in_=rall[:, :]
            )
            for h in range(HPC):
                idx = tci * HPC + h
                rec_b = rpool.tile([64, TCW], F32, tag="recb", name=f"recb{tci}_{h}")
                nc.sync.dma_start(
                    out=rec_b[:, :],
                    in_=rrow[idx:idx + 1, :].to_broadcast([64, TCW]),
                )
                if debug and tci == 0 and h == 0:
                    nc.sync.dma_start(out=d_scr[:, :], in_=rall[0:1, :])
                    nc.sync.dma_start(out=d_rec[:, :], in_=rec_b[:, :])
                nc.vector.tensor_mul(ctxT[h][:, tsl], cues[h][0:64, :], rec_b[:, :])
            # --- output projection for this t-chunk ---
            for tb in range(4 * tci, 4 * tci + 4):
                po = ps_o.tile([128, 512], F32, tag="O")
                for h in range(HPC):
                    nc.tensor.matmul(
                        po[:, 0:384],
                        lhsT=ctxT[h][:, tb * 128:(tb + 1) * 128],
                        rhs=wo_sb[h][:, :],
                        start=(h == 0),
                        stop=(h == HPC - 1),
                    )
                osb = rpool.tile([128, 384], F32, tag="osb", name=f"osb{tb}")
                nc.any.tensor_copy(out=osb[:, :], in_=po[:, 0:384])
                nc.sync.dma_start(
                    out=out[tb * 128:(tb + 1) * 128, :], in_=osb[:, :]
                )

        if debug:
            for mt, name in enumerate(qk_names):
                nc.sync.dma_start(out=d_qt[mt], in_=qkT[name][:, :])
            nc.sync.dma_start(out=d_v[:, :, :], in_=v_sb[:, :, :])
            for h in range(HPC):
                nc.sync.dma_start(out=d_ctxT[h], in_=ctxT[h][:, :])

    return nc


def get_nc():
    global _CACHED_NC
    if _CACHED_NC is None:
        nc = build_nc()
        nc.finalize()
        _CACHED_NC = nc
    return _CACHED_NC


def make_core_inputs(x, Wq, bq, Wk, bk, Wv, bv, Wo, bo):
    """Host-side shard prep. Returns (in_maps, host_add) where host_add[384]
    is added to every output row (exact fold of bv/bo)."""
    scale = 1.0 / math.sqrt(D)
    assert np.all(bq == 0.0) and np.all(bk == 0.0), "kernel assumes bq=bk=0"
    host_add = (bv.astype(np.float64) @ Wo.astype(np.float64) + bo).astype(np.float32)

    si = np.arange(128)[:, None]
    tj = np.arange(128)[None, :]
    zt = np.zeros((128, 512), dtype=np.float32)
    zt[:, 384:512] = (si <= tj).astype(np.float32)

    in_maps = []
    for core in range(NCORES):
        b = core // 2
        h0 = HPC * (core % 2)  # first head (0 or 3)
        cs = slice(h0 * D, (h0 + HPC) * D)
        wq_s = (Wq[:, cs] * scale).astype(np.float32)
        wk_s = Wk[:, cs].astype(np.float32)
        wqk = np.concatenate(
            [
                wq_s[:, 0:128],
                wk_s[:, 0:128],
                np.tile(wq_s[:, 128:192], (1, 2)),
                np.tile(wk_s[:, 128:192], (1, 2)),
            ],
            axis=1,
        )
        wv_p = np.zeros((C, 256), dtype=np.float32)
        wv_p[:, 0:192] = Wv[:, cs]
        in_maps.append(
            {
                "xt": np.ascontiguousarray(x[b].T).astype(np.float32),
                "wqk": np.ascontiguousarray(wqk),
                "wv": wv_p,
                "wo": np.ascontiguousarray(Wo[cs, :]).astype(np.float32),
                "zt": zt,
                "vones": np.ones((128, NTB * HPC * 2), dtype=np.float32),
            }
        )
    return in_maps, host_add


def kernel(x, Wq, bq, Wk, bk, Wv, bv, Wo, bo, _trace=False):
    x = np.asarray(x, dtype=np.float32)
    Wq, bq = np.asarray(Wq, np.float32), np.asarray(bq, np.float32)
    Wk, bk = np.asarray(Wk, np.float32), np.asarray(bk, np.float32)
    Wv, bv = np.asarray(Wv, np.float32), np.asarray(bv, np.float32)
    Wo, bo = np.asarray(Wo, np.float32), np.asarray(bo, np.float32)

    from concourse.bass_utils import run_bass_kernel_spmd

    nc = get_nc()
    in_maps, host_add = make_core_inputs(x, Wq, bq, Wk, bk, Wv, bv, Wo, bo)
    res = run_bass_kernel_spmd(
        nc, in_maps, core_ids=list(range(NCORES)), trace=_trace
    )
    out = np.empty((B, T, C), dtype=np.float32)
    for b in range(B):
        out[b] = res.results[2 * b]["out"] + res.results[2 * b + 1]["out"] + host_add
    if _trace:
        return out, res
    return out


# revision 27
# speedup vs baseline: 1.1691x; 1.1691x over previous
"""Causal multi-head attention (B=4, T=2048, C=384, 6 heads of 64) on 8 trn2 cores.

Sharding: 24 (batch, head) pairs -> 8 cores; core c handles batch c//2 and
heads [3*(c%2), 3*(c%2)+3). Each core computes q/k/v projections for its 3
heads, causal softmax(q k^T / 8) v, and a PARTIAL output projection
ctx_heads @ Wo_heads. Host sums the two partials per batch and adds the
exactly-folded bias terms (bv @ Wo + bo; softmax weights sum to 1 so a v-bias
contributes bv @ Wo to every row).

Requires bq == bk == 0 (true for this problem: spec fill=zeros).
"""

import math
import os
from contextlib import ExitStack

import ml_dtypes
import numpy as np

BF16NP = ml_dtypes.bfloat16

B, T, C = 4, 2048, 384
NH, D = 6, 64          # total heads, head dim
HPC = 3                # heads per core
NCORES = 8
NKC = C // 128         # 3 contraction chunks for the projections
NTB = T // 128         # 16 row blocks
TCW = 512              # t-chunk width for the attention loop
NTC = T // TCW         # 4 t-chunks

_CACHED_NC = None


def build_nc():
    import concourse.bass as bass
    import concourse.mybir as mybir
    import concourse.tile as tile
    from concourse import bacc

    F32 = mybir.dt.float32
    BF16 = mybir.dt.bfloat16
    EXPF = mybir.ActivationFunctionType.Exp

    nc = bacc.Bacc("TRN2", target_bir_lowering=False, debug=False)

    xt = nc.dram_tensor("xt", [C, T], BF16, kind="ExternalInput")
    wqk = nc.dram_tensor("wqk", [C, 512], BF16, kind="ExternalInput")
    wv = nc.dram_tensor("wv", [C, 256], BF16, kind="ExternalInput")
    wo = nc.dram_tensor("wo", [HPC * D, 384], BF16, kind="ExternalInput")
    zt = nc.dram_tensor("zt", [128, 512], BF16, kind="ExternalInput")
    vones = nc.dram_tensor("vones", [128, NTB * HPC * 2], BF16, kind="ExternalInput")
    out = nc.dram_tensor("out", [T, C], F32, kind="ExternalOutput")
    srow = nc.dram_tensor("srow", [NTC * HPC, TCW], F32)  # sums rows scratch
    rrow = nc.dram_tensor("rrow", [NTC * HPC, TCW], F32)  # recip rows scratch
    debug = bool(os.environ.get("KBG_DEBUG"))
    if debug:
        d_qt = nc.dram_tensor("d_qt", [4, 128, T], BF16, kind="ExternalOutput")
        d_v = nc.dram_tensor("d_v", [128, NTB, 3 * 66], BF16, kind="ExternalOutput")
        d_e = nc.dram_tensor("d_e", [128, 1024], BF16, kind="ExternalOutput")
        d_eraw = nc.dram_tensor("d_eraw", [128, 1024], BF16, kind="ExternalOutput")
        d_ctxT = nc.dram_tensor("d_ctxT", [HPC, 64, T], BF16, kind="ExternalOutput")
        d_rec = nc.dram_tensor("d_rec", [64, TCW], F32, kind="ExternalOutput")
        d_cps = nc.dram_tensor("d_cps", [66, TCW], F32, kind="ExternalOutput")
        d_scr = nc.dram_tensor("d_scr", [1, TCW], F32, kind="ExternalOutput")

    with ExitStack() as ctx:
        tc = ctx.enter_context(tile.TileContext(nc))
        const = ctx.enter_context(tc.tile_pool(name="const", bufs=1))
        xpool = ctx.enter_context(tc.tile_pool(name="xp", bufs=1))
        qkpool = ctx.enter_context(tc.tile_pool(name="qkp", bufs=1))
        vpool = ctx.enter_context(tc.tile_pool(name="vp", bufs=1))
        expp = ctx.enter_context(tc.tile_pool(name="expp", bufs=4))
        cxp = ctx.enter_context(tc.tile_pool(name="cxp", bufs=1))
        rpool = ctx.enter_context(tc.tile_pool(name="rp", bufs=4))
        ps_s = ctx.enter_context(tc.tile_pool(name="ps_s", bufs=2, space="PSUM"))
        ps_c = ctx.enter_context(tc.tile_pool(name="ps_c", bufs=3, space="PSUM"))
        ps_o = ctx.enter_context(tc.tile_pool(name="ps_o", bufs=1, space="PSUM"))

        # ---- constants ----
        wqk_sb = []
        wv_sb = []
        for kc in range(NKC):
            w1 = const.tile([128, 512], BF16, tag=f"wqk{kc}")
            nc.sync.dma_start(out=w1, in_=wqk[kc * 128:(kc + 1) * 128, :])
            wqk_sb.append(w1)
            w2 = const.tile([128, 256], BF16, tag=f"wv{kc}")
            nc.sync.dma_start(out=w2, in_=wv[kc * 128:(kc + 1) * 128, :])
            wv_sb.append(w2)
        wo_sb = []
        for h in range(HPC):
            w3 = const.tile([64, 384], BF16, tag=f"wo{h}")
            nc.sync.dma_start(out=w3, in_=wo[h * 64:(h + 1) * 64, :])
            wo_sb.append(w3)
        zt_sb = const.tile([128, 512], BF16, tag="zt")
        nc.sync.dma_start(out=zt_sb, in_=zt[:, :])

        # ---- x^T in SBUF, as 12 [128, 512] chunks for fine-grained deps ----
        xts = [[None] * 4 for _ in range(NKC)]
        for kc in range(NKC):
            for nch in range(4):
                t_ = xpool.tile([128, 512], BF16, tag=f"xt{kc}_{nch}")
                nc.sync.dma_start(
                    out=t_,
                    in_=xt[kc * 128:(kc + 1) * 128, nch * 512:(nch + 1) * 512],
                )
                xts[kc][nch] = t_

        # ---- projections: qT/kT packed [d(2 heads), T] ----
        # wqk columns: mt0=[q0|q1] mt1=[k0|k1] mt2=[q2|q2] mt3=[k2|k2]
        # (q columns pre-scaled by 1/sqrt(D) on host)
        qk_names = ["qT01", "kT01", "qT22", "kT22"]
        qkT = {}
        for mt, name in enumerate(qk_names):
            qkT[name] = qkpool.tile([128, T], BF16, tag=name, name=name)
        for mt, name in enumerate(qk_names):
            for nch in range(4):
                ps = ps_s.tile([128, 1024], F32, tag="S")
                for kc in range(NKC):
                    nc.tensor.matmul(
                        ps[:, 0:512],
                        lhsT=wqk_sb[kc][:, mt * 128:(mt + 1) * 128],
                        rhs=xts[kc][nch][:, :],
                        start=(kc == 0),
                        stop=(kc == NKC - 1),
                    )
                nc.vector.tensor_copy(
                    out=qkT[name][:, nch * 512:(nch + 1) * 512], in_=ps[:, 0:512]
                )

        # ---- v in natural [s, d'] layout + ones column per head (65 wide) ----
        v_sb = vpool.tile([128, NTB, 3 * 66], BF16, tag="vsb")
        ones_view = v_sb.rearrange("p s (h e) -> p s h e", e=66)[:, :, :, 64:66]
        nc.sync.dma_start(
            out=ones_view, in_=vones.rearrange("p (s h o) -> p s h o", h=HPC, o=2)
        )
        for tb in range(NTB):
            ps = ps_s.tile([128, 1024], F32, tag="S")
            for kc in range(NKC):
                nc.tensor.matmul(
                    ps[:, 0:256],
                    lhsT=xts[kc][tb // 4][:, (tb % 4) * 128:(tb % 4 + 1) * 128],
                    rhs=wv_sb[kc][:, :],
                    start=(kc == 0),
                    stop=(kc == NKC - 1),
                )
            dst = v_sb[:, tb, :].rearrange("p (h e) -> p h e", e=66)[:, :, 0:64]
            src = ps[:, 0:192].rearrange("p (h e) -> p h e", e=64)
            nc.vector.tensor_copy(out=dst, in_=src)

        # ---- per-head normalized ctx^T [64, T] ----
        ctxT = [
            cxp.tile([64, T], BF16, tag=f"ctxT{h}", name=f"ctxT{h}")
            for h in range(HPC)
        ]

        def mask_exp_block(e_tile, col0, sbm):
            """Causal-mask the exp'd S^T block at e_tile[:, col0:col0+512].

            sbm = s_block_start - t_chunk_start (>= 0 on diagonal blocks).
            Cols [col0, col0+sbm) are fully above the diagonal (-> *0) and
            cols [col0+sbm, col0+sbm+128) are triangular; zt_sb is laid out
            as [384 zeros | 128-wide triangle] so one suffix slice covers
            both regions in a single multiply."""
            w = sbm + 128
            nc.vector.tensor_mul(
                e_tile[:, col0:col0 + w],
                e_tile[:, col0:col0 + w],
                zt_sb[:, 512 - w:512],
            )

        for tci in range(NTC):
            tsl = slice(tci * TCW, (tci + 1) * TCW)
            nsb = 4 * tci + 4
            cps = [
                ps_c.tile([128, TCW], F32, tag="ctx", name=f"cps{tci}_{h}")
                for h in range(HPC)
            ]
            for sb in range(nsb):
                sbm = sb * 128 - tci * TCW  # >= 0 on diagonal blocks
                # --- heads 0,1: row-tiled pair (K=64 each) ---
                s01 = ps_s.tile([128, 1024], F32, tag="S")
                e01 = expp.tile([128, 1024], BF16, tag="E")
                for hh in range(2):
                    psl = slice(hh * 64, (hh + 1) * 64)
                    nc.tensor.matmul(
                        s01[:, hh * 512:(hh + 1) * 512],
                        lhsT=qkT["kT01"][psl, sb * 128:(sb + 1) * 128],
                        rhs=qkT["qT01"][psl, tsl],
                        start=True,
                        stop=True,
                    )
                nc.scalar.activation(e01[:, :], s01[:, :], EXPF)
                if debug and tci == 0 and sb == 0:
                    nc.sync.dma_start(out=d_eraw[:, :], in_=e01[:, :])
                if sbm >= 0:
                    mask_exp_block(e01, 0, sbm)
                    mask_exp_block(e01, 512, sbm)
                if debug and tci == 0 and sb == 0:
                    nc.sync.dma_start(out=d_e[:, :], in_=e01[:, :])
                for hh in range(2):
                    nc.tensor.matmul(
                        cps[hh][0:66, :],
                        lhsT=v_sb[:, sb, hh * 66:(hh + 1) * 66],
                        rhs=e01[:, hh * 512:(hh + 1) * 512],
                        start=(sb == 0),
                        stop=(sb == nsb - 1),
                    )
                # --- head 2: packed across (sb, sb+1) pairs ---
                if sb % 2 == 0:
                    s2 = ps_s.tile([128, 1024], F32, tag="S")
                    e2 = expp.tile([128, 1024], BF16, tag="E")
                    for j in range(2):
                        sbj = sb + j
                        psl = slice(j * 64, (j + 1) * 64)
                        nc.tensor.matmul(
                            s2[:, j * 512:(j + 1) * 512],
                            lhsT=qkT["kT22"][psl, sbj * 128:(sbj + 1) * 128],
                            rhs=qkT["qT22"][psl, tsl],
                            start=True,
                            stop=True,
                        )
                    nc.scalar.activation(e2[:, :], s2[:, :], EXPF)
                    for j in range(2):
                        sbmj = (sb + j) * 128 - tci * TCW
                        if sbmj >= 0:
                            mask_exp_block(e2, j * 512, sbmj)
                    for j in range(2):
                        nc.tensor.matmul(
                            cps[2][0:66, :],
                            lhsT=v_sb[:, sb + j, 132:198],
                            rhs=e2[:, j * 512:(j + 1) * 512],
                            start=(sb + j == 0),
                            stop=(sb + j == nsb - 1),
                        )
            # --- normalize: ctxT_n = ctx / sums (sums = row 64 of cps) ---
            if debug and tci == 0:
                cpy = rpool.tile([66, TCW], F32, tag="dcpy", name=f"dcpy{tci}")
                nc.vector.tensor_copy(out=cpy[:, :], in_=cps[0][0:66, :])
                nc.sync.dma_start(out=d_cps[:, :], in_=cpy[:, :])
            # --- normalize: evac unnormalized ctx+sums, batched reciprocal ---
            cues = []
            for h in range(HPC):
                cue = rpool.tile([66, TCW], F32, tag=f"cue{h}", name=f"cue{tci}_{h}")
                nc.vector.tensor_copy(out=cue[:, :], in_=cps[h][0:66, :])
                nc.sync.dma_start(
                    out=srow[tci * HPC + h:tci * HPC + h + 1, :], in_=cue[64:65, :]
                )
                cues.append(cue)
            sall = rpool.tile([HPC, TCW], F32, tag="sall", name=f"sall{tci}")
            nc.sync.dma_start(
                out=sall[:, :], in_=srow[tci * HPC:(tci + 1) * HPC, :]
            )
            rall = rpool.tile([HPC, TCW], F32, tag="rall", name=f"rall{tci}")
            nc.vector.reciprocal(out=rall[:, :], in_=sall[:, :])
            nc.sync.dma_start(
                out=rrow[tci * HPC:(tci + 1) * HPC, :], in_=rall[:, :]
            )
            for h in range(HPC):
                idx = tci * HPC + h
                rec_b = rpool.tile([64, TCW], F32, tag="recb", name=f"recb{tci}_{h}")
                nc.sync.dma_start(
                    out=rec_b[:, :],
                    in_=rrow[idx:idx + 1, :].to_broadcast([64, TCW]),
                )
                if debug and tci == 0 and h == 0:
                    nc.sync.dma_start(out=d_scr[:, :], in_=rall[0:1, :])
                    nc.sync.dma_start(out=d_rec[:, :], in_=rec_b[:, :])
                nc.vector.tensor_mul(ctxT[h][:, tsl], cues[h][0:64, :], rec_b[:, :])
            # --- output projection for this t-chunk ---
            for tb in range(4 * tci, 4 * tci + 4):
                po = ps_o.tile([128, 512], F32, tag="O")
                for h in range(HPC):
                    nc.tensor.matmul(
                        po[:, 0:384],
                        lhsT=ctxT[h][:, tb * 128:(tb + 1) * 128],
                        rhs=wo_sb[h][:, :],
                        start=(h == 0),
                        stop=(h == HPC - 1),
                    )
                osb = rpool.tile([128, 384], F32, tag="osb", name=f"osb{tb}")
                nc.any.tensor_copy(out=osb[:, :], in_=po[:, 0:384])
                nc.sync.dma_start(
                    out=out[tb * 128:(tb + 1) * 128, :], in_=osb[:, :]
                )

        if debug:
            for mt, name in enumerate(qk_names):
                nc.sync.dma_start(out=d_qt[mt], in_=qkT[name][:, :])
            nc.sync.dma_start(out=d_v[:, :, :], in_=v_sb[:, :, :])
            for h in range(HPC):
                nc.sync.dma_start(out=d_ctxT[h], in_=ctxT[h][:, :])

    return nc


def get_nc():
    global _CACHED_NC
    if _CACHED_NC is None:
        nc = build_nc()
        nc.finalize()
        _CACHED_NC = nc
    return _CACHED_NC


def make_core_inputs(x, Wq, bq, Wk, bk, Wv, bv, Wo, bo):
    """Host-side shard prep. Returns (in_maps, host_add) where host_add[384]
    is added to every output row (exact fold of bv/bo)."""
    scale = 1.0 / math.sqrt(D)
    assert np.all(bq == 0.0) and np.all(bk == 0.0), "kernel assumes bq=bk=0"
    host_add = (bv.astype(np.float64) @ Wo.astype(np.float64) + bo).astype(np.float32)

    si = np.arange(128)[:, None]
    tj = np.arange(128)[None, :]
    zt = np.zeros((128, 512), dtype=np.float32)
    zt[:, 384:512] = (si <= tj).astype(np.float32)

    in_maps = []
    for core in range(NCORES):
        b = core // 2
        h0 = HPC * (core % 2)  # first head (0 or 3)
        cs = slice(h0 * D, (h0 + HPC) * D)
        wq_s = (Wq[:, cs] * scale).astype(np.float32)
        wk_s = Wk[:, cs].astype(np.float32)
        wqk = np.concatenate(
            [
                wq_s[:, 0:128],
                wk_s[:, 0:128],
                np.tile(wq_s[:, 128:192], (1, 2)),
                np.tile(wk_s[:, 128:192], (1, 2)),
            ],
            axis=1,
        )
        wv_p = np.zeros((C, 256), dtype=np.float32)
        wv_p[:, 0:192] = Wv[:, cs]
        in_maps.append(
            {
                "xt": np.ascontiguousarray(x[b].T).astype(BF16NP),
                "wqk": np.ascontiguousarray(wqk).astype(BF16NP),
                "wv": wv_p.astype(BF16NP),
                "wo": np.ascontiguousarray(Wo[cs, :]).astype(BF16NP),
                "zt": zt.astype(BF16NP),
                "vones": np.ones((128, NTB * HPC * 2), dtype=BF16NP),
            }
        )
    return in_maps, host_add


def kernel(x, Wq, bq, Wk, bk, Wv, bv, Wo, bo, _trace=False):
    x = np.asarray(x, dtype=np.float32)
    Wq, bq = np.asarray(Wq, np.float32), np.asarray(bq, np.float32)
    Wk, bk = np.asarray(Wk, np.float32), np.asarray(bk, np.float32)
    Wv, bv = np.asarray(Wv, np.float32), np.asarray(bv, np.float32)
    Wo, bo = np.asarray(Wo, np.float32), np.asarray(bo, np.float32)

    from concourse.bass_utils import run_bass_kernel_spmd

    nc = get_nc()
    in_maps, host_add = make_core_inputs(x, Wq, bq, Wk, bk, Wv, bv, Wo, bo)
    res = run_bass_kernel_spmd(
        nc, in_maps, core_ids=list(range(NCORES)), trace=_trace
    )
    out = np.empty((B, T, C), dtype=np.float32)
    for b in range(B):
        out[b] = res.results[2 * b]["out"] + res.results[2 * b + 1]["out"] + host_add
    if _trace:
        return out, res
    return out
